# revision 6
# baseline (speedup 1.0000x reference)
"""DLRM DotInteraction kernel for Trainium2 (Bass/Tile), 8-core data parallel.

Problem: dense_feature [B=16384, D=128] f32, sparse_stack [S=26, B, D] f32.
cat = [dense; sparse] per sample -> [B, N=27, D]; G_b = cat_b @ cat_b^T;
out = [dense | tril(G_b) (378 vals, row-major incl diag)] -> [B, 506] f32.

Per core (B_c = 2048 samples = 16 tiles of 128), supertiles of TPS tiles:
  1. HWDGE f32 loads into natural layout nat[s, j, d] (SP queue only).
  2. TensorE f32 transpose of each feature slab -> PSUM, cast f32->f16 on
     the DVE copy to SBUF xt[d, j, s].
  3. TensorE Gram per sample: 4 col-tiled matmuls per group (tile_position
     (0,32c)), K=128, M=N=27, f16 in, f32 PSUM [32c+i, q, j].
  4. Full-width ACT copies PSUM -> gcol[(c,i) part, g, tp, j] (all 128
     partitions per copy).
  5. Flatten: 27 DMAs per supertile write tri rows for all samples straight
     to HBM `out` (read: partition-strided gcol, write: 3-dim DRAM AP).
     Dense passthrough is a single HBM->HBM DMA.
"""

import numpy as np

import concourse.bacc as bacc
import concourse.mybir as mybir
import concourse.tile as tile
from concourse import bass_utils
from concourse.masks import make_identity

B = 16384
D = 128
S = 26
N = S + 1  # 27
NCORES = 8
BC = B // NCORES  # 2048 samples per core
PT = 128  # samples per sbuf tile
GPR = 16  # groups per psum round
TRI = N * (N + 1) // 2  # 378
W = D + TRI  # 506
TPS = 8  # tiles per supertile

f32 = mybir.dt.float32
f16 = mybir.dt.float16


def build_kernel(b_core: int = BC, reps: int = 1):
    nc = bacc.Bacc("TRN2", target_bir_lowering=False, debug=False)
    dense = nc.dram_tensor("dense", [b_core, D], f32, kind="ExternalInput").ap()
    sparse = nc.dram_tensor("sparse", [S, b_core, D], f32, kind="ExternalInput").ap()
    out = nc.dram_tensor("out", [b_core, W], f32, kind="ExternalOutput").ap()

    t_total = b_core // PT
    gpt = PT // 4  # 32 groups per tile
    rpt = gpt // GPR  # psum rounds per tile
    tps = min(TPS, t_total)
    n_super = t_total // tps

    with tile.TileContext(nc) as tc:
        with (
            tc.tile_pool(name="singles", bufs=1) as singles,
            tc.tile_pool(name="nat", bufs=5) as nat_pool,
            tc.tile_pool(name="xt", bufs=4) as xt_pool,
            tc.tile_pool(name="gcol", bufs=2) as gcol_pool,
            tc.tile_pool(name="psum", bufs=3, space="PSUM") as psum_pool,
            tc.tile_pool(name="psumt", bufs=4, space="PSUM") as psumt_pool,
        ):
            id32 = singles.tile([128, 128], f32, name="id32")
            make_identity(nc, id32)

            for _rep in range(reps):
                # dense passthrough: single HBM->HBM DMA
                nc.scalar.dma_start(out=out[:, 0:D], in_=dense[:, :])
                for st in range(n_super):
                    # gcol[32c+i, g, tp, j] = Gram[i,j] of sample 32c+g in
                    # tile tp of this supertile.
                    gcol = gcol_pool.tile([128, gpt, tps, N], f32)

                    for tp in range(tps):
                        t = st * tps + tp
                        rows = slice(t * PT, (t + 1) * PT)
                        # --- load f32, natural layout [s, j, d] ---
                        nat = nat_pool.tile([128, N, D], f32)
                        nc.sync.dma_start(out=nat[:, 0, :], in_=dense[rows, :])
                        nc.sync.dma_start(
                            out=nat[:, 1:N, :],
                            in_=sparse[:, rows, :].rearrange("s b d -> b s d"),
                        )

                        # --- TensorE f32 transpose of each feature slab;
                        # f32->f16 cast happens on the PSUM->SBUF copy ---
                        xt = xt_pool.tile([128, N, PT], f16)
                        for k in range(7):  # 4-slab packs: 6*4 + 3
                            j0 = 4 * k
                            nj = min(4, N - j0)
                            pt_ = psumt_pool.tile([128, 4, PT], f32, tag="pt")
                            for jj in range(nj):
                                nc.tensor.transpose(
                                    pt_[:, jj, :], nat[:, j0 + jj, :], id32
                                )
                            cp = nc.vector.tensor_copy if k % 2 == 0 else nc.scalar.copy
                            cp(out=xt[:, j0 : j0 + nj, :], in_=pt_[:, 0:nj, :])

                        # --- Gram matmuls ---
                        for r in range(rpt):
                            ps = psum_pool.tile([128, GPR, N], f32)
                            for q in range(GPR):
                                g_local = r * GPR + q
                                for c in range(4):
                                    s_local = 32 * c + g_local
                                    nc.tensor.matmul(
                                        out=ps[32 * c : 32 * c + N, q, :],
                                        lhsT=xt[:, :, s_local],
                                        rhs=xt[:, :, s_local],
                                        start=True,
                                        stop=True,
                                        tile_position=(0, 32 * c),
                                    )
                            off = r * GPR
                            # full-width copy: partitions (c,i) all at once;
                            # lanes 32c+27..32c+31 carry garbage, never read.
                            nc.scalar.copy(
                                out=gcol[:, off : off + GPR, tp, :],
                                in_=ps[:, :, :],
                            )

                    # --- flatten: tri row i of every sample straight to HBM.
                    # read partition 32c+i (stride-32 partition dim), free
                    # (g, tp, j); write (cg, tp, j) over DRAM rows. ---
                    ost = out[st * tps * PT : (st + 1) * tps * PT, :].rearrange(
                        "(tp cg) w -> cg tp w", tp=tps
                    )
                    for i in range(N):
                        toff = D + i * (i + 1) // 2
                        nc.scalar.dma_start(
                            out=ost[:, :, toff : toff + i + 1],
                            in_=gcol[i : 97 + i + 1 : 32, :, :, 0 : i + 1],
                        )

    nc.compile()
    return nc


_CACHE: dict = {}


def _get_nc():
    if "nc" not in _CACHE:
        _CACHE["nc"] = build_kernel(BC)
    return _CACHE["nc"]


def kernel(dense_feature, sparse_stack, **run_kwargs):
    dense_feature = np.asarray(dense_feature, dtype=np.float32)
    sparse_stack = np.asarray(sparse_stack, dtype=np.float32)
    assert dense_feature.shape == (B, D)
    assert sparse_stack.shape == (S, B, D)

    nc = run_kwargs.pop("nc", None) or _get_nc()
    in_maps = []
    for ci in range(NCORES):
        sl = slice(ci * BC, (ci + 1) * BC)
        in_maps.append(
            {
                "dense": np.ascontiguousarray(dense_feature[sl]),
                "sparse": np.ascontiguousarray(sparse_stack[:, sl, :]),
            }
        )
    res = bass_utils.run_bass_kernel_spmd(
        nc, in_maps, core_ids=list(range(NCORES)), **run_kwargs
    )
    out = np.concatenate([r["out"] for r in res.results], axis=0)
    if run_kwargs:
        _CACHE["last_result"] = res
    return out


# revision 15
# speedup vs baseline: 2.3392x; 2.3392x over previous
"""DLRM DotInteraction kernel for Trainium2 (Bass/Tile), 8-core data parallel.

Problem: dense_feature [B=16384, D=128] f32, sparse_stack [S=26, B, D] f32.
cat = [dense; sparse] per sample -> [B, N=27, D]; G_b = cat_b @ cat_b^T;
out = [dense | tril(G_b) (378 vals, row-major incl diag)] -> [B, 506] f32.

Per core (B_c = 2048 samples = 16 tiles of 128), default config ("stream"):
  1. SWDGE cast-DMA loads f32 HBM -> f16 SBUF, natural layout nat[s, j, d].
  2. TensorE f16 transpose of each feature slab -> PSUM, DVE/ACT copy to
     SBUF xt[d, j, s].
  3. TensorE Gram per sample: 4 col-tiled matmuls per group (tile_position
     (0,32c)), K=128, M=N=27, f16 in, f32 PSUM [32c+i, q, j].
  4. Full-width ACT copy PSUM -> gcol[(c,i) part, (j, g)] f16 per tile.
  5. DVE 32x32 StreamTranspose swaps (i<->g) within 32-blocks: lands
     samples on partitions (strans[(c,g) part, tp, j, i]) with NO DMA.
  6. Tril-compaction: 27 engine copies per supertile (f16->f32 cast fused),
     then one contiguous 259 KB store per tile (split over both HWDGE rings).
"""

import numpy as np

import concourse.bacc as bacc
import concourse.mybir as mybir
import concourse.tile as tile
from concourse import bass_utils
from concourse.masks import make_identity

B = 16384
D = 128
S = 26
N = S + 1  # 27
NCORES = 8
BC = B // NCORES  # 2048 samples per core
PT = 128  # samples per sbuf tile
GPR = 16  # groups per psum round
TRI = N * (N + 1) // 2  # 378
W = D + TRI  # 506
TPS = 8  # tiles per supertile

f32 = mybir.dt.float32
f16 = mybir.dt.float16


def build_kernel(
    b_core: int = BC,
    reps: int = 1,
    *,
    tps: int = 4,
    flatten: str = "stream",  # "stream" | "rowq" | "hbm" (see below)
    trmode: str = "f16",  # "f32" (PE f32 transpose) | "f16" (cast then f16 transpose)
    nat_bufs: int = 4,
    xt_bufs: int = 4,
    gcol_bufs: int = 4,
    upto: str = "full",  # "load" | "xt" | "gram" | "flat" | "full" (bench ablation)
    flat_split: int = 2,  # flatten DMA queue split: i % flat_split == 0 -> scalar
    store_split: int = 2,  # store queue split: tp % store_split == 0 -> scalar
    shared_lhsT: bool = False,  # timing probe: all gram matmuls share one lhsT
    row_bufs: int = 2,
    gcol_f16: bool = False,  # gcol+tri in f16, engine-cast to f32 before store
    big_store: bool = False,  # one store DMA per supertile instead of per tile
    psum_bufs: int = 4,
    psumt_bufs: int = 4,
):
    nc = bacc.Bacc("TRN2", target_bir_lowering=False, debug=False)
    dense = nc.dram_tensor("dense", [b_core, D], f32, kind="ExternalInput").ap()
    sparse = nc.dram_tensor("sparse", [S, b_core, D], f32, kind="ExternalInput").ap()
    out = nc.dram_tensor("out", [b_core, W], f32, kind="ExternalOutput").ap()

    t_total = b_core // PT
    gpt = PT // 4  # 32 groups per tile
    rpt = gpt // GPR  # psum rounds per tile
    tps = min(tps, t_total)
    n_super = t_total // tps

    with tile.TileContext(nc) as tc:
        with (
            tc.tile_pool(name="singles", bufs=1) as singles,
            tc.tile_pool(name="nat", bufs=nat_bufs) as nat_pool,
            tc.tile_pool(name="xt", bufs=xt_bufs) as xt_pool,
            tc.tile_pool(name="gcol", bufs=gcol_bufs) as gcol_pool,
            tc.tile_pool(name="row", bufs=row_bufs) as row_pool,
            tc.tile_pool(name="psum", bufs=psum_bufs, space="PSUM") as psum_pool,
            tc.tile_pool(name="psumt", bufs=psumt_bufs, space="PSUM") as psumt_pool,
        ):
            id_dt = f32 if trmode == "f32" else f16
            ident = singles.tile([128, 128], id_dt, name="ident")
            make_identity(nc, ident)

            for _rep in range(reps):
                if flatten == "hbm":
                    # dense passthrough: single HBM->HBM DMA
                    nc.scalar.dma_start(out=out[:, 0:D], in_=dense[:, :])
                for st in range(n_super):
                    # gcol[32c+i, g, tp, j] = Gram[i,j] of sample 32c+g in
                    # tile tp of this supertile.
                    gdt = f16 if gcol_f16 else f32
                    if flatten not in ("rect", "stream"):
                        gcol = gcol_pool.tile([128, gpt, tps, N], gdt)
                    if flatten == "stream":
                        rowq = row_pool.tile([128, tps, W], f32)
                        strans = row_pool.tile([128, tps, N, 32], f16, tag="strans")
                    elif flatten == "rowq":
                        rowq = row_pool.tile([128, tps, W], f32)
                        if gcol_f16:
                            rowt = row_pool.tile([128, tps, TRI], f16, tag="rowt")
                    elif flatten == "rect":
                        rowq = row_pool.tile([128, tps, W], f32)
                        rowt27 = row_pool.tile([128, tps, N, N], f16, tag="rowt27")

                    for tp in range(tps):
                        t = st * tps + tp
                        rows = slice(t * PT, (t + 1) * PT)
                        # --- load f32, natural layout [s, j, d] ---
                        nat = nat_pool.tile([128, N, D], f32)
                        if trmode == "f32":
                            nc.sync.dma_start(out=nat[:, 0, :], in_=dense[rows, :])
                            nc.sync.dma_start(
                                out=nat[:, 1:N, :],
                                in_=sparse[:, rows, :].rearrange("s b d -> b s d"),
                            )
                            trin = nat
                        else:
                            # SWDGE cast-DMA load straight to f16
                            nat16 = nat_pool.tile([128, N, D], f16, tag="nat16")
                            nc.gpsimd.dma_start(out=nat16[:, 0, :], in_=dense[rows, :])
                            nc.gpsimd.dma_start(
                                out=nat16[:, 1:N, :],
                                in_=sparse[:, rows, :].rearrange("s b d -> b s d"),
                            )
                            trin = nat16
                        if flatten in ("rowq", "rect", "stream") and upto in ("flat", "full"):
                            nc.sync.dma_start(out=rowq[:, tp, 0:D], in_=dense[rows, :])
                        if upto == "load":
                            continue

                        # --- TensorE transpose of each feature slab; for f32
                        # input the f32->f16 cast happens on the PSUM copy ---
                        xt = xt_pool.tile([128, N, PT], f16)
                        for k in range(7):  # 4-slab packs: 6*4 + 3
                            j0 = 4 * k
                            nj = min(4, N - j0)
                            pt_ = psumt_pool.tile([128, 4, PT], id_dt, tag="pt")
                            for jj in range(nj):
                                nc.tensor.transpose(
                                    pt_[:, jj, :], trin[:, j0 + jj, :], ident
                                )
                            cp = nc.vector.tensor_copy if k % 2 == 0 else nc.scalar.copy
                            cp(out=xt[:, j0 : j0 + nj, :], in_=pt_[:, 0:nj, :])

                        # --- Gram matmuls ---
                        if upto == "xt":
                            continue
                        if flatten == "rect":
                            gcol = gcol_pool.tile([128, gpt, N], gdt, tag="gct")
                        elif flatten == "stream":
                            gcol = gcol_pool.tile([128, N, gpt], f16, tag="gcs")
                        for r in range(rpt):
                            ps = psum_pool.tile([128, GPR, N], f32)
                            for q in range(GPR):
                                g_local = r * GPR + q
                                for c in range(4):
                                    s_local = 32 * c + g_local
                                    lhs_s = 0 if shared_lhsT else s_local
                                    nc.tensor.matmul(
                                        out=ps[32 * c : 32 * c + N, q, :],
                                        lhsT=xt[:, :, lhs_s],
                                        rhs=xt[:, :, s_local],
                                        start=True,
                                        stop=True,
                                        tile_position=(0, 32 * c),
                                    )
                            off = r * GPR
                            # full-width copy: partitions (c,i) all at once;
                            # lanes 32c+27..32c+31 carry garbage, never read.
                            if flatten == "rect":
                                nc.scalar.copy(
                                    out=gcol[:, off : off + GPR, :], in_=ps[:, :, :]
                                )
                            elif flatten == "stream":
                                # (j, g) free layout, g innermost for the
                                # 32x32 stream transpose
                                nc.scalar.copy(
                                    out=gcol[:, :, off : off + GPR],
                                    in_=ps[:, :, :].rearrange("p q j -> p j q"),
                                )
                            else:
                                nc.scalar.copy(
                                    out=gcol[:, off : off + GPR, tp, :],
                                    in_=ps[:, :, :],
                                )

                        # --- stream transpose: 32x32 block transpose on DVE
                        # lands samples on partitions: strans[32c+g, tp, j, i]
                        if flatten == "stream" and upto in ("flat", "full"):
                            nc.vector.transpose(
                                out=strans[:, tp, :, :], in_=gcol[:, :, :]
                            )

                        # --- rect gather: partition transpose (c,i)->(c,g)
                        # per c-block, evenly spread reads ---
                        if flatten == "rect" and upto in ("flat", "full"):
                            for c in range(4):
                                geng = nc.sync if (tp * 4 + c) % 2 else nc.scalar
                                geng.dma_start(
                                    out=rowt27[32 * c : 32 * c + 32, tp, :, :],
                                    in_=gcol[32 * c : 32 * c + N, :, :].rearrange(
                                        "i g j -> g i j"
                                    ),
                                )

                    # --- flatten: tri row i of every sample; read partition
                    # 32c+i (stride-32 partition dim), free (g, tp, j) ---
                    if upto in ("load", "xt", "gram"):
                        continue
                    if flatten == "stream":
                        # tril-compact strans -> rowq with f16->f32 cast
                        for i in range(N):
                            toff = D + i * (i + 1) // 2
                            cpc = nc.vector.tensor_copy if i % 2 else nc.scalar.copy
                            cpc(
                                out=rowq[:, :, toff : toff + i + 1],
                                in_=strans[:, :, 0 : i + 1, i],
                            )
                        if upto == "full":
                            for tp in range(tps):
                                t = st * tps + tp
                                seng = nc.scalar if tp % store_split == 0 else nc.sync
                                seng.dma_start(
                                    out=out[t * PT : (t + 1) * PT, :],
                                    in_=rowq[:, tp, :],
                                )
                    elif flatten == "rect":
                        # tril-compact rowt27 -> rowq with f16->f32 cast
                        for i in range(N):
                            toff = D + i * (i + 1) // 2
                            cpc = nc.vector.tensor_copy if i % 2 else nc.scalar.copy
                            cpc(
                                out=rowq[:, :, toff : toff + i + 1],
                                in_=rowt27[:, :, i, 0 : i + 1],
                            )
                        if upto == "full":
                            for tp in range(tps):
                                t = st * tps + tp
                                seng = nc.scalar if tp % store_split == 0 else nc.sync
                                seng.dma_start(
                                    out=out[t * PT : (t + 1) * PT, :],
                                    in_=rowq[:, tp, :],
                                )
                    elif flatten == "hbm":
                        ost = out[st * tps * PT : (st + 1) * tps * PT, :].rearrange(
                            "(tp cg) w -> cg tp w", tp=tps
                        )
                        for i in range(N):
                            toff = D + i * (i + 1) // 2
                            nc.scalar.dma_start(
                                out=ost[:, :, toff : toff + i + 1],
                                in_=gcol[i : 97 + i + 1 : 32, :, :, 0 : i + 1],
                            )
                    else:
                        for i in range(N):
                            toff = D + i * (i + 1) // 2
                            eng = nc.scalar if i % flat_split == 0 else nc.sync
                            if gcol_f16:
                                toff_t = toff - D
                                eng.dma_start(
                                    out=rowt[:, :, toff_t : toff_t + i + 1],
                                    in_=gcol[i : 97 + i + 1 : 32, :, :, 0 : i + 1],
                                )
                            else:
                                eng.dma_start(
                                    out=rowq[:, :, toff : toff + i + 1],
                                    in_=gcol[i : 97 + i + 1 : 32, :, :, 0 : i + 1],
                                )
                        if gcol_f16:
                            # cast tri f16 -> f32 into rowq (DVE/ACT split)
                            for tp in range(tps):
                                cpc = nc.vector.tensor_copy if tp % 2 else nc.scalar.copy
                                cpc(
                                    out=rowq[:, tp, D:W],
                                    in_=rowt[:, tp, :],
                                )
                        if upto == "full":
                            if big_store:
                                ost = out[
                                    st * tps * PT : (st + 1) * tps * PT, :
                                ].rearrange("(tp cg) w -> cg tp w", tp=tps)
                                nc.scalar.dma_start(out=ost, in_=rowq[:, :, :])
                            else:
                                for tp in range(tps):
                                    t = st * tps + tp
                                    seng = nc.scalar if tp % store_split == 0 else nc.sync
                                    seng.dma_start(
                                        out=out[t * PT : (t + 1) * PT, :],
                                        in_=rowq[:, tp, :],
                                    )

    nc.compile()
    return nc


_CACHE: dict = {}


def _get_nc():
    if "nc" not in _CACHE:
        _CACHE["nc"] = build_kernel(BC)
    return _CACHE["nc"]


def kernel(dense_feature, sparse_stack, **run_kwargs):
    dense_feature = np.asarray(dense_feature, dtype=np.float32)
    sparse_stack = np.asarray(sparse_stack, dtype=np.float32)
    assert dense_feature.shape == (B, D)
    assert sparse_stack.shape == (S, B, D)

    nc = run_kwargs.pop("nc", None) or _get_nc()
    in_maps = []
    for ci in range(NCORES):
        sl = slice(ci * BC, (ci + 1) * BC)
        in_maps.append(
            {
                "dense": np.ascontiguousarray(dense_feature[sl]),
                "sparse": np.ascontiguousarray(sparse_stack[:, sl, :]),
            }
        )
    res = bass_utils.run_bass_kernel_spmd(
        nc, in_maps, core_ids=list(range(NCORES)), **run_kwargs
    )
    out = np.concatenate([r["out"] for r in res.results], axis=0)
    if run_kwargs:
        _CACHE["last_result"] = res
    return out
